# revision 11
# baseline (speedup 1.0000x reference)
"""Single-head causal attention for Trainium2, batch-parallel over 8 NeuronCores.

Reference computation (per batch element b):
    q = x @ Wq + bq; k = x @ Wk + bk; v = x @ Wv + bv        # [T, H]
    s = q @ k.T / sqrt(H); causal mask; w = softmax(s)
    out = w @ v                                              # [T, H]

Shapes: x [8, 2048, 1024] f32, W* [1024, 128], b* [128]. Output [8, 2048, 128].

Strategy: one batch element per core (pure data parallel, no collectives).
The data path runs in bf16 (inputs converted host-side; rel tolerance is
2e-2, measured error ~1e-3):

  - x and W ship as bf16. x.T lands in SBUF via DMA-transpose (XBAR) loads
    straight from DRAM - no PE transposes, no PSUM traffic, no drain copies.
  - Q.T/K.T [H, T] are computed as W.T @ x.T (contract E on partitions).
    V is computed directly in [t, h] block layout (lhsT = x.T chunk slice,
    rhs = Wv chunk), so the AV matmul needs no V transpose. V's bias is a
    broadcast matrix built with one 1-partition matmul.
  - Scores are computed transposed, S.T[k, q] = (K.T chunk).T @ Q.T, only
    over the causal lower triangle at 128x512 block granularity. exp is
    fused on ACT (scale baked in); the diagonal gets one triangular mask
    multiply on DVE.
  - The AV product is computed directly in [q, h] layout: lhsT = P.T block
    column-slice (q-tile), rhs = V block. Output accumulates per q-tile in
    PSUM column slices - no output transposes, and the softmax divide is a
    plain per-partition tensor_scalar on the way out. Row sums accumulate
    per q-tile via ones-vector matmuls so each q-tile's normalization chain
    can retire early.
"""

import sys

if "/opt/trn_rl_repo" not in sys.path:
    sys.path.insert(0, "/opt/trn_rl_repo")

import numpy as np
import ml_dtypes

import concourse.bacc as bacc
import concourse.mybir as mybir
import concourse.tile as tile
from concourse.bass_utils import run_bass_kernel_spmd

F32 = mybir.dt.float32
BF16 = mybir.dt.bfloat16
AF = mybir.ActivationFunctionType

B, T, E, H = 8, 2048, 1024, 128
NE = E // 128  # 8 e-chunks
NT = T // 128  # 16 t-tiles
NG = T // 512  # 4 q-groups
SCALE = 1.0 / float(np.sqrt(H))


def _emit(nc, tc, x, wb, cb, cf, out):
    with (
        tc.tile_pool(name="const", bufs=1) as cpool,
        tc.tile_pool(name="wpool", bufs=1) as wpool,
        tc.tile_pool(name="pers", bufs=1) as pers,
        tc.tile_pool(name="ptp", bufs=1) as ptp,
        tc.tile_pool(name="small", bufs=1) as smallp,
        tc.tile_pool(name="psum", bufs=1, space="PSUM") as psp,
    ):
        # --- constants + weights (scalar/ACT queue; x loads own the SP queue)
        cbt = cpool.tile([128, 386], BF16)
        nc.scalar.dma_start(cbt[:], cb[:, :])
        cft = cpool.tile([128, 3], F32)
        nc.scalar.dma_start(cft[:], cf[:, :])
        ones_col = cbt[:, 0:1]        # bf16 ones [128,1] (rs lhsT)
        tri = cbt[:, 1:129]           # bf16 upper-tri (keep k<=q in [k,q])
        bv_row = cbt[0:1, 129:257]    # bf16 bv as a row
        ones_row = cbt[0:1, 257:385]  # bf16 ones row
        bq_col = cft[:, 0:1]
        bk_col = cft[:, 1:2]
        one_f32 = cft[0:1, 2:3]       # f32 identity scalar for prs transpose

        # Weights in two DMAs (few, large transfers: every DMA's issue is
        # gated on its DMAHW-lane predecessor's full completion, so DMA
        # count is the scarce resource, not bytes).
        wt = wpool.tile([128, NE * 3 * H], BF16)
        for half in range(2):
            rows = slice(half * 512, (half + 1) * 512)
            nc.scalar.dma_start(
                wt[:, half * 4 * 3 * H : (half + 1) * 4 * 3 * H].rearrange(
                    "p (eb c) -> p eb c", eb=4
                ),
                wb[rows, :].rearrange("(eb p) c -> p eb c", p=128),
            )

        def w_chunk(eb, i):
            return wt[:, eb * 3 * H + i * H : eb * 3 * H + (i + 1) * H]

        # --- x.T via DMA-transpose loads (XBAR): quarter panels for the
        # first two panels (finer arrival for startup), halves for the rest.
        XT = [pers.tile([128, T], BF16, tag=f"xt{e}", name=f"xt{e}") for e in range(NE)]

        def load_xt(lo, hi):
            for e in range(NE):
                nc.sync.dma_start_transpose(
                    XT[e][:, lo:hi], x[lo:hi, e * 128 : (e + 1) * 128]
                )

        load_xt(0, 2048)

        QT = pers.tile([128, T], BF16, tag="qt")
        KT = pers.tile([128, T], BF16, tag="kt")
        VN = pers.tile([128, T], BF16, tag="vn")  # V blocks [k,h] at cols 128*kblk
        BVs = smallp.tile([128, 128], F32, tag="bvs")

        # V-bias broadcast matrix: BV[i,j] = bv[j] via 1-partition matmul.
        bvps = psp.tile([128, 128], F32, tag="vac", bufs=2, name="bvps")
        nc.tensor.matmul(bvps[:], lhsT=ones_row, rhs=bv_row, start=True, stop=True)
        nc.scalar.copy(BVs[:], bvps[:])

        def stage_proj(p):
            """Projections for panel p (512 t's): Q.T/K.T cols, V [t,h] blocks."""
            cols = slice(p * 512, (p + 1) * 512)
            ppq = psp.tile([128, 512], F32, tag="qk", bufs=3, name=f"ppq{p}")
            ppk = psp.tile([128, 512], F32, tag="qk", bufs=3, name=f"ppk{p}")
            vacc = psp.tile([128, 512], F32, tag="vac", bufs=2, name=f"vacc{p}")
            for eb in range(NE):
                st, sp = eb == 0, eb == NE - 1
                nc.tensor.matmul(ppq[:], lhsT=w_chunk(eb, 0),
                                 rhs=XT[eb][:, cols], start=st, stop=sp)
                nc.tensor.matmul(ppk[:], lhsT=w_chunk(eb, 1),
                                 rhs=XT[eb][:, cols], start=st, stop=sp)
                for ti in range(4):
                    tsl = slice(p * 512 + ti * 128, p * 512 + (ti + 1) * 128)
                    # start marks the whole 2KB bank pending-zero; only the
                    # first slice sets it, later slices write-first into
                    # their still-pending bytes and then accumulate.
                    nc.tensor.matmul(
                        vacc[:, ti * 128 : (ti + 1) * 128],
                        lhsT=XT[eb][:, tsl], rhs=w_chunk(eb, 2),
                        start=(st and ti == 0), stop=(sp and ti == 3),
                        skip_group_check=True,
                    )
            nc.vector.tensor_scalar_add(QT[:, cols], ppq[:], bq_col)
            nc.vector.tensor_scalar_add(KT[:, cols], ppk[:], bk_col)
            for ti in range(4):
                csl = slice((p * 4 + ti) * 128, (p * 4 + ti + 1) * 128)
                nc.vector.tensor_add(
                    VN[:, csl], vacc[:, ti * 128 : (ti + 1) * 128], BVs[:]
                )

        def stage_attn(g):
            """Attention q-group g (512 q's), causal over kblk 0..4g+3."""
            qlo = g * 512
            nk = 4 * g + 4
            pso = psp.tile([128, 512], F32, tag="ot", bufs=2, name=f"ot{g}")
            rs_ps = psp.tile([1, 512], F32, tag="rs", bufs=1, name=f"rs{g}")
            rs_row = smallp.tile([1, 512], F32, tag="rs_row", bufs=2,
                                 name=f"rsrow{g}")
            for kblk in range(nk):
                j = kblk - 4 * g  # >=0 on the diagonal panel
                off = max(j, 0) * 128
                pst = psp.tile([128, 512], F32, tag="qk", bufs=3,
                               name=f"st{g}_{kblk}")
                nc.tensor.matmul(
                    pst[:, off:],
                    lhsT=KT[:, kblk * 128 : (kblk + 1) * 128],
                    rhs=QT[:, qlo + off : qlo + 512],
                    start=True, stop=True,
                )
                pt = ptp.tile([128, 512], BF16, tag=f"pt{kblk}",
                              name=f"pt{g}_{kblk}")
                nc.scalar.activation(pt[:, off:], pst[:, off:], AF.Exp, scale=SCALE)
                if j >= 0:
                    dsl = slice(j * 128, (j + 1) * 128)
                    nc.vector.tensor_mul(pt[:, dsl], pt[:, dsl], tri)
                for qt in range(4):
                    if j > qt:
                        continue
                    qsl = slice(qt * 128, (qt + 1) * 128)
                    last = kblk == 4 * g + qt
                    nc.tensor.matmul(
                        pso[:, qsl], lhsT=pt[:, qsl], rhs=VN[:, kblk * 128 : (kblk + 1) * 128],
                        start=(kblk == 0 and qt == 0), stop=last,
                        skip_group_check=True,
                    )
                    nc.tensor.matmul(
                        rs_ps[0:1, qsl], lhsT=ones_col, rhs=pt[:, qsl],
                        start=(kblk == 0 and qt == 0), stop=last,
                        skip_group_check=True,
                    )
                    if last:
                        if qt % 2 == 0:
                            nc.vector.tensor_copy(rs_row[0:1, qsl], rs_ps[0:1, qsl])
                        else:
                            nc.scalar.copy(rs_row[0:1, qsl], rs_ps[0:1, qsl])
            # normalize + store. Early groups store as one [512,128] DMA; the
            # last group splits qt0-2 / qt3 so the tail chain only carries a
            # [128,128] store.
            last_g = g == NG - 1
            obg = smallp.tile([128, 512], F32, tag="obg", bufs=2, name=f"obg{g}")
            for qt in range(4):
                qsl = slice(qt * 128, (qt + 1) * 128)
                prs = psp.tile([128, 1], F32, tag="vac", bufs=2,
                               name=f"prs{g}_{qt}")
                nc.tensor.transpose(prs[:], rs_row[0:1, qsl], one_f32)
                rinv = smallp.tile([128, 1], F32, tag="rinv", bufs=4,
                                   name=f"rinv{g}_{qt}")
                nc.vector.reciprocal(rinv[:], prs[:])
                nc.vector.tensor_scalar_mul(obg[:, qsl], pso[:, qsl], rinv[:])
                if last_g and qt == 2:
                    nc.sync.dma_start(
                        out[qlo : qlo + 384, :].rearrange(
                            "(qt p) h -> p qt h", p=128
                        ),
                        obg[:, 0:384].rearrange("p (qt h) -> p qt h", h=H),
                    )
            if last_g:
                nc.sync.dma_start(out[qlo + 384 : qlo + 512, :], obg[:, 384:512])
            else:
                nc.sync.dma_start(
                    out[qlo : qlo + 512, :].rearrange(
                        "(qt p) h -> p qt h", p=128
                    ),
                    obg[:].rearrange("p (qt h) -> p qt h", h=H),
                )

        stage_proj(0)
        stage_proj(1)
        stage_attn(0)
        stage_proj(2)
        stage_attn(1)
        stage_proj(3)
        stage_attn(2)
        stage_attn(3)


def build_program():
    nc = bacc.Bacc("TRN2", target_bir_lowering=False, debug=False)
    x = nc.dram_tensor("x", [T, E], BF16, kind="ExternalInput").ap()
    wb = nc.dram_tensor("wb", [E, 3 * H], BF16, kind="ExternalInput").ap()
    cb = nc.dram_tensor("cb", [128, 386], BF16, kind="ExternalInput").ap()
    cf = nc.dram_tensor("cf", [128, 3], F32, kind="ExternalInput").ap()
    out = nc.dram_tensor("out", [T, H], F32, kind="ExternalOutput").ap()
    with tile.TileContext(nc) as tc:
        _emit(nc, tc, x, wb, cb, cf, out)
    nc.compile()
    return nc


_program = None


def _get_program():
    global _program
    if _program is None:
        _program = build_program()
    return _program


def make_in_maps(x, Wq, Wk, Wv, bq, bk, bv):
    bf = ml_dtypes.bfloat16
    x = np.asarray(x, np.float32).astype(bf)
    wb = np.ascontiguousarray(
        np.concatenate(
            [np.asarray(w, np.float32) for w in (Wq, Wk, Wv)], axis=1
        ).astype(bf)
    )
    cb = np.zeros((128, 386), dtype=bf)
    cb[:, 0] = bf(1.0)
    cb[:, 1:129] = np.triu(np.ones((128, 128), np.float32)).astype(bf)
    cb[0, 129:257] = np.asarray(bv, np.float32).reshape(H).astype(bf)
    cb[0, 257:385] = bf(1.0)
    cf = np.zeros((128, 3), dtype=np.float32)
    cf[:, 0] = np.asarray(bq, np.float32).reshape(H)
    cf[:, 1] = np.asarray(bk, np.float32).reshape(H)
    cf[0, 2] = 1.0
    return [
        {"x": np.ascontiguousarray(x[b]), "wb": wb, "cb": cb, "cf": cf}
        for b in range(B)
    ]


def kernel(x, Wq, Wk, Wv, bq, bk, bv):
    nc = _get_program()
    in_maps = make_in_maps(x, Wq, Wk, Wv, bq, bk, bv)
    res = run_bass_kernel_spmd(nc, in_maps, list(range(B)))
    return np.stack([res.results[b]["out"] for b in range(B)], axis=0).astype(
        np.float32
    )


# revision 15
# speedup vs baseline: 1.0415x; 1.0415x over previous
"""Single-head causal attention for Trainium2, batch-parallel over 8 NeuronCores.

Reference computation (per batch element b):
    q = x @ Wq + bq; k = x @ Wk + bk; v = x @ Wv + bv        # [T, H]
    s = q @ k.T / sqrt(H); causal mask; w = softmax(s)
    out = w @ v                                              # [T, H]

Shapes: x [8, 2048, 1024] f32, W* [1024, 128], b* [128]. Output [8, 2048, 128].

Strategy: one batch element per core (pure data parallel, no collectives).
The data path runs in bf16 (inputs converted host-side; rel tolerance is
2e-2, measured error ~1e-3):

  - x and W ship as bf16. x.T lands in SBUF via DMA-transpose (XBAR) loads
    straight from DRAM - no PE transposes, no PSUM traffic, no drain copies.
  - Q.T/K.T [H, T] are computed as W.T @ x.T (contract E on partitions).
    V is computed directly in [t, h] block layout (lhsT = x.T chunk slice,
    rhs = Wv chunk), so the AV matmul needs no V transpose. V's bias is a
    broadcast matrix built with one 1-partition matmul.
  - Scores are computed transposed, S.T[k, q] = (K.T chunk).T @ Q.T, only
    over the causal lower triangle at 128x512 block granularity. exp is
    fused on ACT (scale baked in); the diagonal gets one triangular mask
    multiply on DVE.
  - The AV product is computed directly in [q, h] layout: lhsT = P.T block
    column-slice (q-tile), rhs = V block. Output accumulates per q-tile in
    PSUM column slices - no output transposes, and the softmax divide is a
    plain per-partition tensor_scalar on the way out. Row sums accumulate
    per q-tile via ones-vector matmuls so each q-tile's normalization chain
    can retire early.
"""

import sys

if "/opt/trn_rl_repo" not in sys.path:
    sys.path.insert(0, "/opt/trn_rl_repo")

import numpy as np
import ml_dtypes

import concourse.bacc as bacc
import concourse.mybir as mybir
import concourse.tile as tile
from concourse.bass_utils import run_bass_kernel_spmd

F32 = mybir.dt.float32
BF16 = mybir.dt.bfloat16
AF = mybir.ActivationFunctionType

B, T, E, H = 8, 2048, 1024, 128
NE = E // 128  # 8 e-chunks
NT = T // 128  # 16 t-tiles
NG = T // 512  # 4 q-groups
SCALE = 1.0 / float(np.sqrt(H))


def _emit(nc, tc, x, wb, cb, cf, out):
    with (
        tc.tile_pool(name="const", bufs=1) as cpool,
        tc.tile_pool(name="wpool", bufs=1) as wpool,
        tc.tile_pool(name="pers", bufs=1) as pers,
        tc.tile_pool(name="ptp", bufs=1) as ptp,
        tc.tile_pool(name="small", bufs=1) as smallp,
        tc.tile_pool(name="psum", bufs=1, space="PSUM") as psp,
    ):
        # All startup DMAs go on the single SP queue in need-order: DMAs
        # issued close together on BOTH queues form serial completion
        # chains (each waits its DMAHW-lane predecessor's full completion),
        # so one queue + few large transfers issues cleanly back-to-back.
        cbt = cpool.tile([128, 386], BF16)
        cft = cpool.tile([128, 3], F32)
        ones_col = cbt[:, 0:1]        # bf16 ones [128,1] (rs lhsT)
        tri = cbt[:, 1:129]           # bf16 upper-tri (keep k<=q in [k,q])
        bv_row = cbt[0:1, 129:257]    # bf16 bv as a row
        ones_row = cbt[0:1, 257:385]  # bf16 ones row
        bq_col = cft[:, 0:1]
        bk_col = cft[:, 1:2]
        one_f32 = cft[0:1, 2:3]       # f32 identity scalar for prs transpose

        wt = wpool.tile([128, NE * 3 * H], BF16)

        def load_w(half):
            rows = slice(half * 512, (half + 1) * 512)
            nc.sync.dma_start(
                wt[:, half * 4 * 3 * H : (half + 1) * 4 * 3 * H].rearrange(
                    "p (eb c) -> p eb c", eb=4
                ),
                wb[rows, :].rearrange("(eb p) c -> p eb c", p=128),
            )

        def w_chunk(eb, i):
            return wt[:, eb * 3 * H + i * H : eb * 3 * H + (i + 1) * H]

        # x.T via DMA-transpose loads (XBAR), one full [2048,128]->[128,2048]
        # e-chunk per DMA, interleaved with the weight halves and constants.
        XT = [pers.tile([128, T], BF16, tag=f"xt{e}", name=f"xt{e}") for e in range(NE)]

        def load_xt(e):
            nc.sync.dma_start_transpose(XT[e][:, :], x[:, e * 128 : (e + 1) * 128])

        load_xt(0)
        load_w(0)
        load_xt(1)
        load_w(1)
        nc.sync.dma_start(cbt[:], cb[:, :])
        nc.sync.dma_start(cft[:], cf[:, :])
        for e in range(2, NE):
            load_xt(e)

        QT = pers.tile([128, T], BF16, tag="qt")
        KT = pers.tile([128, T], BF16, tag="kt")
        VN = pers.tile([128, T], BF16, tag="vn")  # V blocks [k,h] at cols 128*kblk
        BVs = smallp.tile([128, 128], F32, tag="bvs")

        # V-bias broadcast matrix: BV[i,j] = bv[j] via 1-partition matmul.
        bvps = psp.tile([128, 128], F32, tag="vac", bufs=2, name="bvps")
        nc.tensor.matmul(bvps[:], lhsT=ones_row, rhs=bv_row, start=True, stop=True)
        nc.scalar.copy(BVs[:], bvps[:])

        def stage_proj(pa, pb):
            """Projections for panels pa, pb, interleaved per e-chunk so the
            tensor engine consumes each chunk DMA fully on arrival."""
            acc = {}
            for p in (pa, pb):
                acc[p] = (
                    psp.tile([128, 512], F32, tag="qk", bufs=4, name=f"ppq{p}"),
                    psp.tile([128, 512], F32, tag="qk", bufs=4, name=f"ppk{p}"),
                    psp.tile([128, 512], F32, tag="vac", bufs=2, name=f"vacc{p}"),
                )
            for eb in range(NE):
                st, sp = eb == 0, eb == NE - 1
                for p in (pa, pb):
                    cols = slice(p * 512, (p + 1) * 512)
                    ppq, ppk, vacc = acc[p]
                    nc.tensor.matmul(ppq[:], lhsT=w_chunk(eb, 0),
                                     rhs=XT[eb][:, cols], start=st, stop=sp)
                    nc.tensor.matmul(ppk[:], lhsT=w_chunk(eb, 1),
                                     rhs=XT[eb][:, cols], start=st, stop=sp)
                    for ti in range(4):
                        tsl = slice(p * 512 + ti * 128, p * 512 + (ti + 1) * 128)
                        # start marks the whole 2KB bank pending-zero; only
                        # the first slice sets it, later slices write-first
                        # into their still-pending bytes and then accumulate.
                        nc.tensor.matmul(
                            vacc[:, ti * 128 : (ti + 1) * 128],
                            lhsT=XT[eb][:, tsl], rhs=w_chunk(eb, 2),
                            start=(st and ti == 0), stop=(sp and ti == 3),
                            skip_group_check=True,
                        )
            for p in (pa, pb):
                cols = slice(p * 512, (p + 1) * 512)
                ppq, ppk, vacc = acc[p]
                nc.vector.tensor_scalar_add(QT[:, cols], ppq[:], bq_col)
                nc.vector.tensor_scalar_add(KT[:, cols], ppk[:], bk_col)
                for ti in range(4):
                    csl = slice((p * 4 + ti) * 128, (p * 4 + ti + 1) * 128)
                    nc.vector.tensor_add(
                        VN[:, csl], vacc[:, ti * 128 : (ti + 1) * 128], BVs[:]
                    )

        def stage_attn(g):
            """Attention q-group g (512 q's), causal over kblk 0..4g+3."""
            qlo = g * 512
            nk = 4 * g + 4
            pso = psp.tile([128, 512], F32, tag="ot", bufs=1, name=f"ot{g}")
            rs_ps = psp.tile([1, 512], F32, tag="rs", bufs=1, name=f"rs{g}")
            rs_row = smallp.tile([1, 512], F32, tag="rs_row", bufs=2,
                                 name=f"rsrow{g}")
            for kblk in range(nk):
                j = kblk - 4 * g  # >=0 on the diagonal panel
                off = max(j, 0) * 128
                pst = psp.tile([128, 512], F32, tag="qk", bufs=4,
                               name=f"st{g}_{kblk}")
                nc.tensor.matmul(
                    pst[:, off:],
                    lhsT=KT[:, kblk * 128 : (kblk + 1) * 128],
                    rhs=QT[:, qlo + off : qlo + 512],
                    start=True, stop=True,
                )
                pt = ptp.tile([128, 512], BF16, tag=f"pt{kblk}",
                              name=f"pt{g}_{kblk}")
                nc.scalar.activation(pt[:, off:], pst[:, off:], AF.Exp, scale=SCALE)
                if j >= 0:
                    dsl = slice(j * 128, (j + 1) * 128)
                    nc.vector.tensor_mul(pt[:, dsl], pt[:, dsl], tri)
                for qt in range(4):
                    if j > qt:
                        continue
                    qsl = slice(qt * 128, (qt + 1) * 128)
                    last = kblk == 4 * g + qt
                    nc.tensor.matmul(
                        pso[:, qsl], lhsT=pt[:, qsl], rhs=VN[:, kblk * 128 : (kblk + 1) * 128],
                        start=(kblk == 0 and qt == 0), stop=last,
                        skip_group_check=True,
                    )
                    nc.tensor.matmul(
                        rs_ps[0:1, qsl], lhsT=ones_col, rhs=pt[:, qsl],
                        start=(kblk == 0 and qt == 0), stop=last,
                        skip_group_check=True,
                    )
                    if last:
                        if qt % 2 == 0:
                            nc.vector.tensor_copy(rs_row[0:1, qsl], rs_ps[0:1, qsl])
                        else:
                            nc.scalar.copy(rs_row[0:1, qsl], rs_ps[0:1, qsl])
            # normalize + store. Early groups store as one [512,128] DMA; the
            # last group splits qt0-2 / qt3 so the tail chain only carries a
            # [128,128] store.
            last_g = g == NG - 1
            obg = smallp.tile([128, 512], F32, tag="obg", bufs=2, name=f"obg{g}")
            for qt in range(4):
                qsl = slice(qt * 128, (qt + 1) * 128)
                prs = psp.tile([128, 1], F32, tag="vac", bufs=2,
                               name=f"prs{g}_{qt}")
                nc.tensor.transpose(prs[:], rs_row[0:1, qsl], one_f32)
                rinv = smallp.tile([128, 1], F32, tag="rinv", bufs=4,
                                   name=f"rinv{g}_{qt}")
                nc.vector.reciprocal(rinv[:], prs[:])
                nc.vector.tensor_scalar_mul(obg[:, qsl], pso[:, qsl], rinv[:])
                if last_g and qt == 2:
                    nc.sync.dma_start(
                        out[qlo : qlo + 384, :].rearrange(
                            "(qt p) h -> p qt h", p=128
                        ),
                        obg[:, 0:384].rearrange("p (qt h) -> p qt h", h=H),
                    )
            if last_g:
                nc.sync.dma_start(out[qlo + 384 : qlo + 512, :], obg[:, 384:512])
            else:
                nc.sync.dma_start(
                    out[qlo : qlo + 512, :].rearrange(
                        "(qt p) h -> p qt h", p=128
                    ),
                    obg[:].rearrange("p (qt h) -> p qt h", h=H),
                )

        stage_proj(0, 1)
        stage_attn(0)
        stage_proj(2, 3)
        stage_attn(1)
        stage_attn(2)
        stage_attn(3)


def build_program():
    nc = bacc.Bacc("TRN2", target_bir_lowering=False, debug=False)
    x = nc.dram_tensor("x", [T, E], BF16, kind="ExternalInput").ap()
    wb = nc.dram_tensor("wb", [E, 3 * H], BF16, kind="ExternalInput").ap()
    cb = nc.dram_tensor("cb", [128, 386], BF16, kind="ExternalInput").ap()
    cf = nc.dram_tensor("cf", [128, 3], F32, kind="ExternalInput").ap()
    out = nc.dram_tensor("out", [T, H], F32, kind="ExternalOutput").ap()
    with tile.TileContext(nc) as tc:
        _emit(nc, tc, x, wb, cb, cf, out)
    nc.compile()
    return nc


_program = None


def _get_program():
    global _program
    if _program is None:
        _program = build_program()
    return _program


def make_in_maps(x, Wq, Wk, Wv, bq, bk, bv):
    bf = ml_dtypes.bfloat16
    x = np.asarray(x, np.float32).astype(bf)
    wb = np.ascontiguousarray(
        np.concatenate(
            [np.asarray(w, np.float32) for w in (Wq, Wk, Wv)], axis=1
        ).astype(bf)
    )
    cb = np.zeros((128, 386), dtype=bf)
    cb[:, 0] = bf(1.0)
    cb[:, 1:129] = np.triu(np.ones((128, 128), np.float32)).astype(bf)
    cb[0, 129:257] = np.asarray(bv, np.float32).reshape(H).astype(bf)
    cb[0, 257:385] = bf(1.0)
    cf = np.zeros((128, 3), dtype=np.float32)
    cf[:, 0] = np.asarray(bq, np.float32).reshape(H)
    cf[:, 1] = np.asarray(bk, np.float32).reshape(H)
    cf[0, 2] = 1.0
    return [
        {"x": np.ascontiguousarray(x[b]), "wb": wb, "cb": cb, "cf": cf}
        for b in range(B)
    ]


def kernel(x, Wq, Wk, Wv, bq, bk, bv):
    nc = _get_program()
    in_maps = make_in_maps(x, Wq, Wk, Wv, bq, bk, bv)
    res = run_bass_kernel_spmd(nc, in_maps, list(range(B)))
    return np.stack([res.results[b]["out"] for b in range(B)], axis=0).astype(
        np.float32
    )
